# revision 1
# baseline (speedup 1.0000x reference)
# Trainium2 Bass kernel for nn_CvtLstm: ConvLSTM cell with 4-branch,
# 4-head spatial attention. Data-parallel over batch N=32 across 8
# NeuronCores (4 samples per core); weights replicated to every core.
#
# Per-core layout: channels on partitions, flattened 16x16 spatial (256)
# on the free dim. conv3x3 = 9 shifted matmuls reading a zero-padded
# [128, 2, 18, 18] tile. Attention scores are computed directly in the
# transposed [d, q] layout (lhsT = per-head k rows, K=32 row-partial
# matmuls); exp on the ACT engine with no max subtraction (scores lie in
# [-9, 8]); the PV product and the softmax denominator Z come from one
# M=64 matmul per (head, d-chunk) whose weight columns are [vT_g | ones];
# normalization is a DMA head-restack + reciprocal + multiply.
#
# Hardware constraint honored throughout: two row-partial matmuls at
# different row groups back-to-back fault the device (LDWEIGHTS pull-ahead
# across non-conflicting row groups). A full-row K>=64... strictly a
# row-range-conflicting matmul between them is safe; emission order is the
# per-engine execution order, so score matmuls are ordered g-outer/c-inner
# with full-row dummy separators at group changes, and iterations are
# separated by the (full-row) PV/Z matmuls of the previous iteration.

import numpy as np

N, I, H, W = 32, 64, 16, 16
R, CM, A, HEADS, HC = 128, 128, 128, 4, 32
HW = H * W           # 256
S = 4                # samples per core
NCORES = 8

_CACHE = {}


def _build_program():
    import contextlib
    import concourse.bacc as bacc
    import concourse.mybir as mybir
    import concourse.tile as tile
    import concourse.bass as bass

    F32 = mybir.dt.float32
    F32R = mybir.dt.float32r
    AF = mybir.ActivationFunctionType

    nc = bacc.Bacc("TRN2", target_bir_lowering=False, debug=False)

    def dram(name, shape, kind="ExternalInput"):
        return nc.dram_tensor(name, list(shape), F32, kind=kind).ap()

    xin = dram("xin", [S, I, HW])
    hin = dram("hin", [S, R, HW])
    cin = dram("cin", [S, R, HW])
    zpad = dram("zpad", [128, 648])
    winTd = dram("winT", [I, R])
    b_ind = dram("b_in", [R, 1])
    wconvTd = dram("wconvT", [128, 2, 9, 128])
    wqkTd = dram("wqkT", [128, 2, 4, 128])
    wvTd = dram("wvT", [128, 2, 256])
    onesd = dram("onesd", [128, 32])
    wtokTd = dram("wtokT", [128, 4, 4, 128])
    btokd = dram("btok", [128, 4])
    wskipTd = dram("wskipT", [128, 4, 2, 128])
    woutTd = dram("woutT", [128, 128])
    boutd = dram("bout", [128, 1])
    yout = dram("yout", [S, R, HW], kind="ExternalOutput")

    QSRC = [0, 0, 1, 1]   # q source per branch: 0=xc, 1=hc
    KSRC = [0, 1, 0, 1]   # k/v source per branch
    BORDER = [3, 1, 2, 0]  # per-pass branch order (b3 = pure hc, earliest)

    with tile.TileContext(nc) as tc:
        with contextlib.ExitStack() as ctx:
            wpool = ctx.enter_context(tc.tile_pool(name="wts", bufs=1))
            sbA = ctx.enter_context(tc.tile_pool(name="sbA", bufs=2))
            sbB = ctx.enter_context(tc.tile_pool(name="sbB", bufs=2))
            stp = ctx.enter_context(tc.tile_pool(name="st", bufs=2, space="PSUM"))
            azp = ctx.enter_context(tc.tile_pool(name="az", bufs=1, space="PSUM"))
            pwp = ctx.enter_context(tc.tile_pool(name="pw", bufs=2, space="PSUM"))

            # ---------------- weights to SBUF ----------------
            def wload(name, src, shape, dt=F32R):
                t = wpool.tile(shape, dt, tag=name, name=name)
                nc.sync.dma_start(out=t, in_=src.bitcast(dt) if dt == F32R else src)
                return t

            winT_s = wload("winT", winTd, [I, R])
            wconvT_s = wload("wconvT", wconvTd, [128, 2, 9, 128])
            wqkT_s = wload("wqkT", wqkTd, [128, 2, 4, 128])
            wvT_s = wload("wvT", wvTd, [128, 2, 256])
            ones_s = wload("ones", onesd, [128, 32])
            wtokT_s = wload("wtokT", wtokTd, [128, 4, 4, 128])
            wskipT_s = wload("wskipT", wskipTd, [128, 4, 2, 128])
            woutT_s = wload("woutT", woutTd, [128, 128])
            b_in_s = wload("b_in", b_ind, [R, 1], F32)
            btok_s = wload("btok", btokd, [128, 4], F32)
            bout_s = wload("bout", boutd, [128, 1], F32)

            # ---------------- per-pass state ----------------
            xc_sb = [None, None]
            hc_sb = [None, None]
            q_sb = [[None] * 4, [None] * 4]
            k_sb = [[None] * 4, [None] * 4]
            vt_sb = [[None] * 4, [None] * 4]
            a_all = [None, None]
            cprev_sb = [None, None]
            gate_sb = [[None] * 4, [None] * 4]

            def emit_input_pads(p):
                """zero-padded xt/h tiles + x2 + XT matmul + tanh."""
                xt_pad = sbA.tile([128, 648], F32R, tag="xtpad", name="xtpad")
                h_pad = sbA.tile([128, 648], F32R, tag="hpad", name="hpad")
                x2 = sbA.tile([64, 2, 256], F32R, tag="x2", name="x2")
                nc.sync.dma_start(out=xt_pad, in_=zpad.bitcast(F32R))
                nc.sync.dma_start(out=h_pad, in_=zpad.bitcast(F32R))
                hv = h_pad.rearrange("p (s y x) -> p s y x", s=2, y=18, x=18)
                for s in range(2):
                    nc.sync.dma_start(
                        out=hv[:, s, 1:17, 1:17],
                        in_=hin[2 * p + s].rearrange(
                            "c (h w) -> c h w", h=16).bitcast(F32R))
                nc.sync.dma_start(
                    out=x2,
                    in_=xin[2 * p:2 * p + 2].rearrange("s c q -> c s q").bitcast(F32R))
                XT = pwp.tile([128, 512], F32, tag="pw", name="XT")
                nc.tensor.matmul(out=XT, lhsT=winT_s,
                                 rhs=x2.rearrange("p s q -> p (s q)"),
                                 start=True, stop=True)
                xv = xt_pad.rearrange("p (s y x) -> p s y x", s=2, y=18, x=18)
                nc.scalar.activation(
                    out=xv[:, :, 1:17, 1:17],
                    in_=XT.rearrange("p (s h w) -> p s h w", s=2, h=16, w=16),
                    func=AF.Tanh, bias=b_in_s)
                return xt_pad, h_pad

            def emit_conv(p, src, pad):
                """3x3 SAME conv via 9 shifted matmuls; src 0=xc, 1=hc."""
                CP = pwp.tile([128, 512], F32, tag="pw", name="CP")
                pv = pad.rearrange("p (s y x) -> p s y x", s=2, y=18, x=18)
                for t in range(9):
                    ky, kx = divmod(t, 3)
                    nc.tensor.matmul(out=CP, lhsT=wconvT_s[:, src, t, :],
                                     rhs=pv[:, :, ky:ky + 16, kx:kx + 16],
                                     start=(t == 0), stop=(t == 8))
                dst = sbA.tile([128, 512], F32R, tag=("xc" if src == 0 else "hc"), name=("xc" if src == 0 else "hc"))
                nc.vector.tensor_copy(dst, CP)
                if src == 0:
                    xc_sb[p] = dst
                else:
                    hc_sb[p] = dst

            def emit_qk(p, b):
                srcq = xc_sb[p] if QSRC[b] == 0 else hc_sb[p]
                srck = xc_sb[p] if KSRC[b] == 0 else hc_sb[p]
                QB = pwp.tile([128, 512], F32, tag="pw", name="QB")
                nc.tensor.matmul(out=QB, lhsT=wqkT_s[:, 0, b, :], rhs=srcq,
                                 start=True, stop=True)
                q_sb[p][b] = sbB.tile([128, 512], F32R, tag=f"q{b}", name=f"q{b}")
                nc.vector.tensor_copy(q_sb[p][b], QB)
                KB = pwp.tile([128, 512], F32, tag="pw", name="KB")
                nc.tensor.matmul(out=KB, lhsT=wqkT_s[:, 1, b, :], rhs=srck,
                                 start=True, stop=True)
                k_sb[p][b] = sbB.tile([128, 512], F32R, tag=f"k{b}", name=f"k{b}")
                nc.vector.tensor_copy(k_sb[p][b], KB)

            def emit_vt(p, src):
                """vT for the two branches fed by `src`; fills [vT_g | ones]
                64-wide head blocks of vt_sb[b] = [128, (s c) 4, 256]."""
                b0 = src            # branches (0,2) from xc, (1,3) from hc
                for b in (b0, b0 + 2):
                    if vt_sb[p][b] is None:
                        vt_sb[p][b] = sbB.tile([128, 1024], F32R, tag=f"vt{b}", name=f"vt{b}")
                src_sb = xc_sb[p] if src == 0 else hc_sb[p]
                sv = src_sb.rearrange("p (s c d) -> p s c d", s=2, c=2)
                for s in range(2):
                    for c in range(2):
                        VT = pwp.tile([128, 256], F32, tag="pw", name="VT")
                        nc.tensor.matmul(out=VT, lhsT=sv[:, s, c, :],
                                         rhs=wvT_s[:, src, :],
                                         start=True, stop=True)
                        sc = s * 2 + c
                        for j, b in enumerate((b0, b0 + 2)):
                            dst = vt_sb[p][b][:, sc * 256:(sc + 1) * 256]
                            dst = dst.rearrange("p (g d) -> p g d", g=4)[:, :, 0:32]
                            srcv = VT[:, j * 128:(j + 1) * 128].rearrange(
                                "p (g d) -> p g d", g=4)
                            nc.vector.tensor_copy(dst, srcv)
                for b in (b0, b0 + 2):
                    dst = vt_sb[p][b].rearrange(
                        "p (n d) -> p n d", n=16)[:, :, 32:64]
                    srcap = bass.AP(tensor=onesd.tensor, offset=0,
                                    ap=[[32, 128], [0, 16], [1, 32]])
                    nc.sync.dma_start(out=dst, in_=srcap.bitcast(F32R))

            def emit_cprev(p):
                cprev_sb[p] = sbA.tile([128, 512], F32, tag="cprev", name="cprev")
                nc.sync.dma_start(
                    out=cprev_sb[p],
                    in_=cin[2 * p:2 * p + 2].rearrange("s c q -> c s q"))

            # ---------------- attention iteration pieces ----------------
            def emit_scores_exp(p, b, s):
                """returns pT tile [128, 2048] f32r = exp(scores^T), layout
                (g, c, q) 4x2x256."""
                kv = k_sb[p][b].rearrange("p (s c d) -> p s c d", s=2, c=2)
                qv = q_sb[p][b].rearrange("p (s q) -> p s q", s=2)
                pT = sbB.tile([128, 2048], F32R, tag="pt", name="pT")
                for h in range(2):
                    ST = stp.tile([128, 1024], F32, tag="st", name="ST")
                    for gg in range(2):
                        g = 2 * h + gg
                        if (h, gg) != (0, 0):
                            # full-row dummy separator into a slice the next
                            # score matmul overwrites (start=True clears it)
                            dsl = ST[0:32, 512:544] if gg == 1 else ST[0:32, 0:32]
                            nc.tensor.matmul(out=dsl, lhsT=ones_s,
                                             rhs=ones_s, start=True, stop=True,
                                             skip_group_check=True)
                        for c in range(2):
                            nc.tensor.matmul(
                                out=ST[:, gg * 512 + c * 256:gg * 512 + c * 256 + 256],
                                lhsT=kv[32 * g:32 * g + 32, s, c, :],
                                rhs=qv[32 * g:32 * g + 32, s, :],
                                start=True, stop=True, skip_group_check=True,
                                tile_position=(32 * g, 0))
                    nc.scalar.activation(out=pT[:, h * 1024:(h + 1) * 1024],
                                         in_=ST, func=AF.Exp)
                return pT

            def emit_pvz(p, b, s, pT):
                AZ = azp.tile([64, 1024], F32, tag="az", name="AZ")
                for g in range(4):
                    for c in range(2):
                        sc = s * 2 + c
                        nc.tensor.matmul(
                            out=AZ[0:64, g * 256:(g + 1) * 256],
                            lhsT=vt_sb[p][b][:, sc * 256 + 64 * g:sc * 256 + 64 * g + 64],
                            rhs=pT[:, g * 512 + c * 256:g * 512 + c * 256 + 256],
                            start=(c == 0), stop=(c == 1), skip_group_check=True)
                return AZ

            def emit_norm(p, b, s, AZ):
                a_flat = sbB.tile([64, 1024], F32, tag="aflat", name="a_flat")
                nc.vector.tensor_copy(a_flat, AZ)
                a_tmp = sbB.tile([128, 256], F32R, tag="atmp", name="a_tmp")
                zb = sbB.tile([128, 256], F32, tag="zb", name="zb")
                rz = sbB.tile([128, 256], F32, tag="rz", name="rz")
                for g in range(4):
                    nc.sync.dma_start(
                        out=a_tmp[32 * g:32 * g + 32, :],
                        in_=a_flat[0:32, g * 256:(g + 1) * 256].bitcast(F32R))
                    nc.sync.dma_start(
                        out=zb[32 * g:32 * g + 32, :],
                        in_=a_flat[32:64, g * 256:(g + 1) * 256])
                nc.vector.reciprocal_approx_fast(out=rz, in_=zb)
                if a_all[p] is None:
                    a_all[p] = sbA.tile([128, 2048], F32R, tag="aall", name="a_all")
                slot = b * 2 + s
                nc.vector.tensor_mul(a_all[p][:, slot * 256:(slot + 1) * 256],
                                     a_tmp.bitcast(F32), rz)

            # ---------------- gates / state / output ----------------
            def emit_gate(p, gi):
                G = pwp.tile([128, 512], F32, tag="pw", name="G")
                av = a_all[p].rearrange("p (b s q) -> p b (s q)", b=4, s=2)
                for b in range(4):
                    nc.tensor.matmul(out=G, lhsT=wtokT_s[:, gi, b, :],
                                     rhs=av[:, b, :],
                                     start=(b == 0), stop=False)
                nc.tensor.matmul(out=G, lhsT=wskipT_s[:, gi, 0, :],
                                 rhs=xc_sb[p], start=False, stop=False)
                nc.tensor.matmul(out=G, lhsT=wskipT_s[:, gi, 1, :],
                                 rhs=hc_sb[p], start=False, stop=True)
                gate_sb[p][gi] = sbA.tile([128, 512], F32, tag=f"gate{gi}", name=f"gate{gi}")
                func = AF.Tanh if gi == 2 else AF.Sigmoid
                nc.scalar.activation(out=gate_sb[p][gi], in_=G, func=func,
                                     bias=btok_s[:, gi:gi + 1])

            def emit_update_out(p):
                gi_, gf_, gg_, go_ = gate_sb[p]
                fc = sbA.tile([128, 512], F32, tag="fc", name="fc")
                nc.vector.tensor_mul(fc, gf_, cprev_sb[p])
                ig = sbA.tile([128, 512], F32, tag="ig", name="ig")
                nc.vector.tensor_mul(ig, gi_, gg_)
                cs = sbA.tile([128, 512], F32, tag="c", name="cs")
                nc.vector.tensor_add(cs, fc, ig)
                tcs = sbA.tile([128, 512], F32, tag="tc", name="tcs")
                nc.scalar.activation(out=tcs, in_=cs, func=AF.Tanh)
                hs = sbA.tile([128, 512], F32R, tag="h", name="hs")
                nc.vector.tensor_mul(hs, go_, tcs)
                OUT = pwp.tile([128, 512], F32, tag="pw", name="OUT")
                nc.tensor.matmul(out=OUT, lhsT=woutT_s, rhs=hs,
                                 start=True, stop=True)
                osb = sbA.tile([128, 512], F32, tag="out", name="osb")
                nc.vector.tensor_scalar_add(osb, OUT, bout_s[:, 0:1])
                nc.sync.dma_start(
                    out=yout[2 * p:2 * p + 2].rearrange("s c q -> c s q"),
                    in_=osb.rearrange("p (s q) -> p s q", s=2))

            # ---------------- emission schedule ----------------
            # prologue: pass-0 essentials up to branch 3 (pure hc)
            xt_pad0, h_pad0 = emit_input_pads(0)
            emit_conv(0, 1, h_pad0)      # hc pass0
            emit_qk(0, 3)
            emit_vt(0, 1)                # vT for b1, b3 (hc source)
            pads1 = [None]

            def filler(i):
                if i == 0:
                    emit_conv(0, 0, xt_pad0)          # xc pass0
                elif i == 1:
                    emit_qk(0, 1)
                    emit_qk(0, 2)
                elif i == 2:
                    emit_qk(0, 0)
                    emit_vt(0, 0)
                    emit_cprev(0)
                elif i == 3:
                    pads1[0] = emit_input_pads(1)
                elif i == 4:
                    emit_conv(1, 1, pads1[0][1])      # hc pass1
                elif i == 5:
                    emit_conv(1, 0, pads1[0][0])      # xc pass1
                elif i == 6:
                    emit_qk(1, 3)
                    emit_vt(1, 1)
                elif i == 7:
                    emit_qk(1, 1)
                    emit_qk(1, 2)
                elif i == 8:
                    emit_qk(1, 0)
                    emit_vt(1, 0)
                    emit_cprev(1)
                elif i in (9, 10, 11, 12):
                    emit_gate(0, i - 9)
                elif i == 13:
                    emit_update_out(0)

            iters = [(p, b, s) for p in (0, 1) for b in BORDER for s in (0, 1)]
            prev = None
            for i, (p, b, s) in enumerate(iters):
                pT = emit_scores_exp(p, b, s)
                if prev is not None:
                    pp, pb, ps, ppT = prev
                    AZ = emit_pvz(pp, pb, ps, ppT)
                    emit_norm(pp, pb, ps, AZ)
                prev = (p, b, s, pT)
                filler(i)
            pp, pb, ps, ppT = prev
            AZ = emit_pvz(pp, pb, ps, ppT)
            emit_norm(pp, pb, ps, AZ)
            for gi in range(4):
                emit_gate(1, gi)
            emit_update_out(1)

    nc.compile()
    return nc


def _prep_shared(inputs):
    f = np.float32
    c = np.ascontiguousarray
    W_cx, W_ch = np.asarray(inputs["W_cx"], f), np.asarray(inputs["W_ch"], f)
    W_q, W_k, W_v = (np.asarray(inputs[k], f) for k in ("W_q", "W_k", "W_v"))
    W_tok, W_skip = np.asarray(inputs["W_tok"], f), np.asarray(inputs["W_skip"], f)
    shared = {
        "zpad": np.zeros((128, 648), f),
        "winT": c(np.asarray(inputs["W_in"], f).T),
        "b_in": c(np.asarray(inputs["b_in"], f).reshape(R, 1)),
        # [i, src, tap, o]
        "wconvT": c(np.stack([W_cx.transpose(1, 2, 3, 0).reshape(128, 9, 128),
                              W_ch.transpose(1, 2, 3, 0).reshape(128, 9, 128)],
                             axis=1)),
        # [c, (q|k), b, a]
        "wqkT": c(np.stack([W_q.transpose(2, 0, 1), W_k.transpose(2, 0, 1)],
                           axis=1)),
        # [c, srcpair, a-pair]: xc feeds branches (0,2), hc feeds (1,3)
        "wvT": c(np.stack([
            np.concatenate([W_v[0].T, W_v[2].T], axis=1),
            np.concatenate([W_v[1].T, W_v[3].T], axis=1)], axis=1)),
        "onesd": np.ones((128, 32), f),
        # [a, gate, branch, r]
        "wtokT": c(W_tok.transpose(3, 0, 1, 2)),
        "btok": c(np.asarray(inputs["b_tok"], f).T),
        # [c, gate, src, r]
        "wskipT": c(W_skip.transpose(3, 0, 1, 2)),
        "woutT": c(np.asarray(inputs["W_out"], f).T),
        "bout": c(np.asarray(inputs["b_out"], f).reshape(R, 1)),
    }
    return shared


def kernel(**inputs):
    from concourse.bass_utils import run_bass_kernel_spmd
    if "nc" not in _CACHE:
        _CACHE["nc"] = _build_program()
    nc = _CACHE["nc"]
    f = np.float32
    x = np.asarray(inputs["x"], f).reshape(N, I, HW)
    hp = np.asarray(inputs["h_prev"], f).reshape(N, R, HW)
    cp = np.asarray(inputs["c_prev"], f).reshape(N, R, HW)
    shared = _prep_shared(inputs)
    in_maps = []
    for ci in range(NCORES):
        sl = slice(S * ci, S * ci + S)
        m = dict(shared)
        m["xin"] = np.ascontiguousarray(x[sl])
        m["hin"] = np.ascontiguousarray(hp[sl])
        m["cin"] = np.ascontiguousarray(cp[sl])
        in_maps.append(m)
    res = run_bass_kernel_spmd(nc, in_maps, core_ids=list(range(NCORES)))
    y = np.concatenate([r["yout"].reshape(S, R, H, W) for r in res.results],
                       axis=0)
    return y.astype(np.float32)



# revision 15
# speedup vs baseline: 1.0301x; 1.0301x over previous
# Trainium2 Bass kernel for nn_CvtLstm: ConvLSTM cell with 4-branch,
# 4-head spatial attention. Data-parallel over batch N=32 across 8
# NeuronCores (4 samples per core); weights replicated to every core.
#
# Per-core layout: channels on partitions, flattened 16x16 spatial (256)
# on the free dim. conv3x3 = 9 shifted matmuls reading a zero-padded
# [128, 2, 18, 18] tile. Attention scores are computed directly in the
# transposed [d, q] layout (lhsT = per-head k rows, K=32 row-partial
# matmuls); exp on the ACT engine with no max subtraction (scores lie in
# [-9, 8]); the PV product and the softmax denominator Z come from one
# M=64 matmul per (head, d-chunk) whose weight columns are [vT_g | ones].
# Matmul outputs must start at partition 0, so heads are separated via a
# DMA restack -- but batched over groups of 4 attention iterations (8
# DMAs per group instead of 32) followed by one batched reciprocal and
# one batched multiply over [128, 1024].
#
# Hardware constraint honored throughout: two row-partial matmuls at
# different row groups back-to-back fault the device. Full-row (K=128)
# matmuls between them are safe. Emission order is the per-engine
# execution order, so the previous iteration's full-row PV/Z matmuls are
# interleaved into the 4 row-group boundaries of the current iteration's
# score matmuls (iteration 0 uses 4 tiny full-row dummies instead).
#
# Gate sigmoids are computed as sigmoid(x) = 0.5*(1 + tanh(x/2)) so every
# activation (tanh/exp) lives in one ACT function table -- no table swaps.
# All weights arrive in one packed DRAM blob via 3 chunked DMAs; zero
# padding and the vt ones-columns are written by Pool-engine memsets.
# Elementwise work is spread over Pool/DVE/ACT to keep each under the PE
# roofline.

import numpy as np

N, I, H, W = 32, 64, 16, 16
R, CM, A, HEADS, HC = 128, 128, 128, 4, 32
HW = H * W           # 256
S = 4                # samples per core
NCORES = 8

# weight blob column offsets
OFF_WIN = 0          # [64p, 128] (rows 64:128 zero)
OFF_BIN = 128        # [128, 1]
OFF_CONVH = 129      # [128, 9*128] W_ch taps
OFF_CONVX = OFF_CONVH + 1152   # [128, 9*128] W_cx taps
OFF_QK = OFF_CONVX + 1152      # [128, 2*4*128] (q|k, branch)
OFF_WV = OFF_QK + 1024         # [128, 2*256] (src, branch-pair)
OFF_ONES = OFF_WV + 512        # [128, 32]
OFF_TOK = OFF_ONES + 32        # [128, 4*4*128] (gate, branch)
OFF_SKIP = OFF_TOK + 2048      # [128, 4*2*128] (gate, src)
OFF_WOUT = OFF_SKIP + 1024     # [128, 128]
OFF_BTOK = OFF_WOUT + 128      # [128, 4] (pre-scaled 0.5 except gate 2)
OFF_BOUT = OFF_BTOK + 4        # [128, 1]
NWCOL = OFF_BOUT + 1

# chunk boundaries for the 3 weight DMAs (ordered by first use)
WCH1 = OFF_CONVX            # winT + b_in + conv_h
WCH2 = OFF_TOK              # conv_x + qk + wv + ones
WCH3 = NWCOL                # tok + skip + wout + btok + bout

_CACHE = {}


def _build_program():
    import contextlib
    import concourse.bacc as bacc
    import concourse.mybir as mybir
    import concourse.tile as tile
    import concourse.bass as bass

    F32 = mybir.dt.float32
    F32R = mybir.dt.float32r
    AF = mybir.ActivationFunctionType
    ALU = mybir.AluOpType

    nc = bacc.Bacc("TRN2", target_bir_lowering=False, debug=False)

    def dram(name, shape, kind="ExternalInput"):
        return nc.dram_tensor(name, list(shape), F32, kind=kind).ap()

    xin = dram("xin", [S, I, HW])
    hin = dram("hin", [S, R, HW])
    cin = dram("cin", [S, R, HW])
    wblob = dram("wblob", [128, NWCOL])
    yout = dram("yout", [S, R, HW], kind="ExternalOutput")

    QSRC = [0, 0, 1, 1]   # q source per branch: 0=xc, 1=hc
    KSRC = [0, 1, 0, 1]   # k/v source per branch
    BORDER = [3, 1, 2, 0]  # per-pass branch order (b3 = pure hc, earliest)

    with tile.TileContext(nc) as tc:
        with contextlib.ExitStack() as ctx:
            wpool = ctx.enter_context(tc.tile_pool(name="wts", bufs=1))
            sbA = ctx.enter_context(tc.tile_pool(name="sbA", bufs=2))
            sbB = ctx.enter_context(tc.tile_pool(name="sbB", bufs=2))
            sbN = ctx.enter_context(tc.tile_pool(name="sbN", bufs=1))
            stp = ctx.enter_context(tc.tile_pool(name="st", bufs=2, space="PSUM"))
            azp = ctx.enter_context(tc.tile_pool(name="az", bufs=1, space="PSUM"))
            pwp = ctx.enter_context(tc.tile_pool(name="pw", bufs=2, space="PSUM"))

            # ---------------- weights to SBUF (one tile, 3 DMAs) ----------
            wb = wpool.tile([128, NWCOL], F32R, tag="wb", name="wb")
            nc.sync.dma_start(out=wb[:, 0:WCH1],
                              in_=wblob[:, 0:WCH1].bitcast(F32R))
            nc.sync.dma_start(out=wb[:, WCH1:WCH2],
                              in_=wblob[:, WCH1:WCH2].bitcast(F32R))
            nc.sync.dma_start(out=wb[:, WCH2:WCH3],
                              in_=wblob[:, WCH2:WCH3].bitcast(F32R))

            def wcols(off, n):
                return wb[:, off:off + n]

            ones32 = wcols(OFF_ONES, 32)
            b_in_ap = wcols(OFF_BIN, 1).bitcast(F32)
            btok_ap = wcols(OFF_BTOK, 4).bitcast(F32)
            bout_ap = wcols(OFF_BOUT, 1).bitcast(F32)

            # ---------------- per-pass state ----------------
            pad_sb = [None, None]      # [128, 2, 648]: 0=xt pad, 1=h pad
            xh_sb = [None, None]       # [128, 1024]: xc | hc
            q_sb = [[None] * 4, [None] * 4]
            k_sb = [[None] * 4, [None] * 4]
            vt_sb = [[None] * 4, [None] * 4]   # per branch: [128, 4, 4, 64]
            a_all = [None, None]
            cprev_sb = [None, None]
            gate_sb = [[None] * 4, [None] * 4]

            def emit_input_pads(p):
                """zero-padded xt/h pads (Pool memset) + x2 + XT + tanh."""
                pad = sbA.tile([128, 2, 648], F32R, tag="padc", name="pad")
                pad_sb[p] = pad
                nc.gpsimd.memset(pad.bitcast(F32), 0.0)
                x2 = sbA.tile([64, 2, 256], F32R, tag="x2", name="x2")
                hv = pad[:, 1, :].rearrange("p (s y x) -> p s y x", s=2, y=18, x=18)
                for s in range(2):
                    nc.sync.dma_start(
                        out=hv[:, s, 1:17, 1:17],
                        in_=hin[2 * p + s].rearrange(
                            "c (h w) -> c h w", h=16).bitcast(F32R))
                nc.sync.dma_start(
                    out=x2,
                    in_=xin[2 * p:2 * p + 2].rearrange("s c q -> c s q").bitcast(F32R))
                XT = pwp.tile([128, 512], F32, tag="pw", name="XT")
                nc.tensor.matmul(out=XT, lhsT=wb[0:64, OFF_WIN:OFF_WIN + 128],
                                 rhs=x2.rearrange("p s q -> p (s q)"),
                                 start=True, stop=True)
                xv = pad[:, 0, :].rearrange("p (s y x) -> p s y x", s=2, y=18, x=18)
                nc.scalar.activation(
                    out=xv[:, :, 1:17, 1:17],
                    in_=XT.rearrange("p (s h w) -> p s h w", s=2, h=16, w=16),
                    func=AF.Tanh, bias=b_in_ap)

            def emit_conv(p, src):
                """3x3 SAME conv via 9 shifted matmuls; src 0=xc, 1=hc."""
                CP = pwp.tile([128, 512], F32, tag="pw", name="CP")
                pv = pad_sb[p][:, src, :].rearrange(
                    "p (s y x) -> p s y x", s=2, y=18, x=18)
                woff = OFF_CONVX if src == 0 else OFF_CONVH
                for t in range(9):
                    ky, kx = divmod(t, 3)
                    nc.tensor.matmul(out=CP, lhsT=wcols(woff + t * 128, 128),
                                     rhs=pv[:, :, ky:ky + 16, kx:kx + 16],
                                     start=(t == 0), stop=(t == 8))
                if xh_sb[p] is None:
                    xh_sb[p] = sbA.tile([128, 1024], F32R, tag="xh", name="xh")
                nc.vector.tensor_copy(xh_sb[p][:, src * 512:(src + 1) * 512], CP)

            def emit_qk(p, b):
                srcq = xh_sb[p][:, QSRC[b] * 512:QSRC[b] * 512 + 512]
                srck = xh_sb[p][:, KSRC[b] * 512:KSRC[b] * 512 + 512]
                QB = pwp.tile([128, 512], F32, tag="pw", name="QB")
                nc.tensor.matmul(out=QB, lhsT=wcols(OFF_QK + b * 128, 128),
                                 rhs=srcq, start=True, stop=True)
                q_sb[p][b] = sbB.tile([128, 512], F32R, tag=f"q{b}", name=f"q{b}")
                nc.vector.tensor_copy(q_sb[p][b], QB)
                KB = pwp.tile([128, 512], F32, tag="pw", name="KB")
                nc.tensor.matmul(out=KB, lhsT=wcols(OFF_QK + 512 + b * 128, 128),
                                 rhs=srck, start=True, stop=True)
                k_sb[p][b] = sbB.tile([128, 512], F32R, tag=f"k{b}", name=f"k{b}")
                nc.vector.tensor_copy(k_sb[p][b], KB)

            def emit_vt(p, src):
                """vt[d, (sc), (g), (v32|ones32)] for branches (src, src+2)."""
                for j in range(2):
                    b = src + 2 * j
                    vt_sb[p][b] = sbB.tile([128, 4, 4, 64], F32R,
                                           tag=f"vt{b}", name=f"vt{b}")
                    nc.gpsimd.memset(vt_sb[p][b][:, :, :, 32:64].bitcast(F32), 1.0)
                for s in range(2):
                    for c in range(2):
                        VT = pwp.tile([128, 256], F32, tag="pw", name="VT")
                        nc.tensor.matmul(
                            out=VT,
                            lhsT=xh_sb[p][:, src * 512 + s * 256 + c * 128:
                                          src * 512 + s * 256 + c * 128 + 128],
                            rhs=wcols(OFF_WV + src * 256, 256),
                            start=True, stop=True)
                        vv = VT.rearrange("p (j g d) -> p j g d", j=2, g=4)
                        for j in range(2):
                            b = src + 2 * j
                            nc.vector.tensor_copy(
                                vt_sb[p][b][:, s * 2 + c, :, 0:32], vv[:, j, :, :])

            def emit_cprev(p):
                cprev_sb[p] = sbA.tile([128, 512], F32, tag="cprev", name="cprev")
                nc.sync.dma_start(
                    out=cprev_sb[p],
                    in_=cin[2 * p:2 * p + 2].rearrange("s c q -> c s q"))

            # ---------------- attention iteration pieces ----------------
            def emit_scores_exp(p, b, s, seps):
                """scores + exp; seps = 4 lists of full-row matmul thunks
                emitted at the row-group boundaries (after g0/g1/g2/g3)."""
                kv = k_sb[p][b].rearrange("p (s c d) -> p s c d", s=2, c=2)
                qv = q_sb[p][b].rearrange("p (s q) -> p s q", s=2)
                pT = sbB.tile([128, 2048], F32R, tag="pt", name="pT")
                for h in range(2):
                    ST = stp.tile([128, 1024], F32, tag="st", name="ST")
                    for gg in range(2):
                        g = 2 * h + gg
                        for c in range(2):
                            nc.tensor.matmul(
                                out=ST[:, gg * 512 + c * 256:gg * 512 + c * 256 + 256],
                                lhsT=kv[32 * g:32 * g + 32, s, c, :],
                                rhs=qv[32 * g:32 * g + 32, s, :],
                                start=True, stop=True, skip_group_check=True,
                                tile_position=(32 * g, 0))
                        for t in seps[g]:
                            t()
                    nc.scalar.activation(out=pT[:, h * 1024:(h + 1) * 1024],
                                         in_=ST, func=AF.Exp)
                return pT

            def make_pvz(p, b, s, pT):
                """AZ [64, 1024] = per head g: [a_g; Z_g] in col block g.
                Returns 8 full-row matmul thunks."""
                AZ = azp.tile([64, 1024], F32, tag="az", name="AZ")
                vt = vt_sb[p][b]
                thunks = []
                for g in range(4):
                    for c in range(2):
                        def pvmm(g=g, c=c):
                            nc.tensor.matmul(
                                out=AZ[0:64, g * 256:(g + 1) * 256],
                                lhsT=vt[:, s * 2 + c, g, :],
                                rhs=pT[:, g * 512 + c * 256:g * 512 + c * 256 + 256],
                                start=(c == 0), stop=(c == 1),
                                skip_group_check=True)
                        thunks.append(pvmm)
                return AZ, thunks

            def emit_azcopy(it, AZ, grp_tiles):
                """copy AZ psum into this group's a_flat slice [64, 1024]."""
                grp, slot = it // 4, it % 4
                if grp_tiles[grp] is None:
                    grp_tiles[grp] = sbN.tile([64, 4096], F32, tag="aflat",
                                              name="a_flat")
                af = grp_tiles[grp]
                eng = nc.scalar if it % 4 == 1 else nc.vector
                if eng is nc.scalar:
                    nc.scalar.activation(
                        out=af[:, slot * 1024:(slot + 1) * 1024], in_=AZ,
                        func=AF.Copy)
                else:
                    nc.vector.tensor_copy(
                        af[:, slot * 1024:(slot + 1) * 1024], AZ)

            def emit_group_norm(grp, grp_tiles):
                """restack 4 iterations' a/Z via 8 DMAs, then one batched
                reciprocal (DVE) and one batched multiply (Pool)."""
                p = grp // 2
                af = grp_tiles[grp]
                afv = af.rearrange("p (it g q) -> p it g q", it=4, g=4)
                a_tmp = sbN.tile([128, 4, 256], F32R, tag="atmp", name="a_tmp")
                zb = sbN.tile([128, 4, 256], F32, tag="zb", name="zb")
                for g in range(4):
                    nc.sync.dma_start(
                        out=a_tmp[32 * g:32 * g + 32, :, :],
                        in_=afv[0:32, :, g, :].bitcast(F32R))
                    nc.sync.dma_start(
                        out=zb[32 * g:32 * g + 32, :, :],
                        in_=afv[32:64, :, g, :])
                rz = sbN.tile([128, 4, 256], F32, tag="rz", name="rz")
                nc.vector.reciprocal_approx_fast(out=rz, in_=zb)
                if a_all[p] is None:
                    a_all[p] = sbA.tile([128, 2048], F32R, tag="aall", name="a_all")
                half = grp % 2
                nc.gpsimd.tensor_mul(
                    a_all[p][:, half * 1024:(half + 1) * 1024],
                    a_tmp.rearrange("p it q -> p (it q)").bitcast(F32),
                    rz.rearrange("p it q -> p (it q)"))

            # ---------------- gates / state / output ----------------
            def emit_gate(p, gi):
                # a_all slot order is iteration order: (branch BORDER[j], s)
                G = pwp.tile([128, 512], F32, tag="pw", name="G")
                av = a_all[p].rearrange("p (j s q) -> p j (s q)", j=4, s=2)
                for pos, b in enumerate(BORDER):
                    nc.tensor.matmul(out=G,
                                     lhsT=wcols(OFF_TOK + (gi * 4 + b) * 128, 128),
                                     rhs=av[:, pos, :],
                                     start=(pos == 0), stop=False)
                nc.tensor.matmul(out=G, lhsT=wcols(OFF_SKIP + (gi * 2) * 128, 128),
                                 rhs=xh_sb[p][:, 0:512], start=False, stop=False)
                nc.tensor.matmul(out=G, lhsT=wcols(OFF_SKIP + (gi * 2 + 1) * 128, 128),
                                 rhs=xh_sb[p][:, 512:1024], start=False, stop=True)
                gate_sb[p][gi] = sbA.tile([128, 512], F32, tag=f"gate{gi}",
                                          name=f"gate{gi}")
                scale = 1.0 if gi == 2 else 0.5
                nc.scalar.activation(out=gate_sb[p][gi], in_=G, func=AF.Tanh,
                                     bias=btok_ap[:, gi:gi + 1], scale=scale)

            def emit_update_out(p):
                # pass 0 math on the idle Pool engine, pass 1 on DVE (short tail)
                eng = nc.gpsimd if p == 0 else nc.vector
                ti, tf, gg_, to = gate_sb[p]
                def fixup(t):
                    # sigmoid(x) = 0.5*tanh(x/2) + 0.5; t = tanh(x/2 + b/2)
                    eng.tensor_scalar(out=t, in0=t, scalar1=0.5, scalar2=0.5,
                                      op0=ALU.mult, op1=ALU.add)
                    return t
                i_ = fixup(ti)
                f_ = fixup(tf)
                o_ = fixup(to)
                fc = sbA.tile([128, 512], F32, tag="fc", name="fc")
                eng.tensor_mul(fc, f_, cprev_sb[p])
                ig = sbA.tile([128, 512], F32, tag="ig", name="ig")
                eng.tensor_mul(ig, i_, gg_)
                eng.tensor_add(fc, fc, ig)            # c state
                nc.scalar.activation(out=ig, in_=fc, func=AF.Tanh)
                hs = sbA.tile([128, 512], F32R, tag="hs", name="hs")
                nc.vector.tensor_mul(hs, o_, ig)      # h_new
                OUT = pwp.tile([128, 512], F32, tag="pw", name="OUT")
                nc.tensor.matmul(out=OUT, lhsT=wcols(OFF_WOUT, 128),
                                 rhs=hs, start=True, stop=True)
                nc.vector.tensor_scalar_add(ig, OUT, bout_ap[:, 0:1])
                nc.sync.dma_start(
                    out=yout[2 * p:2 * p + 2].rearrange("s c q -> c s q"),
                    in_=ig.rearrange("p (s q) -> p s q", s=2))

            def dummy_sep():
                # tiny full-row (K=128) matmul used as a row-group separator
                d = pwp.tile([32, 32], F32, tag="pw", name="dsep")
                nc.tensor.matmul(out=d, lhsT=ones32, rhs=ones32,
                                 start=True, stop=True, skip_group_check=True)

            # ---------------- emission schedule ----------------
            # prologue: pass-0 essentials up to branch 3 (pure hc)
            emit_input_pads(0)
            emit_conv(0, 1)              # hc pass0
            emit_qk(0, 3)
            emit_vt(0, 1)                # vT for b1, b3 (hc source)

            def filler(i):
                if i == 0:
                    emit_conv(0, 0)               # xc pass0
                elif i == 1:
                    emit_qk(0, 1)
                    emit_qk(0, 2)
                elif i == 2:
                    emit_qk(0, 0)
                    emit_vt(0, 0)
                    emit_cprev(0)
                elif i == 3:
                    emit_input_pads(1)
                elif i == 4:
                    emit_conv(1, 1)               # hc pass1
                elif i == 5:
                    emit_conv(1, 0)               # xc pass1
                elif i == 6:
                    emit_qk(1, 3)
                    emit_vt(1, 1)
                elif i == 7:
                    emit_qk(1, 1)
                    emit_qk(1, 2)
                elif i == 8:
                    emit_qk(1, 0)
                    emit_vt(1, 0)
                    emit_cprev(1)
                elif i in (9, 10, 11, 12):
                    emit_gate(0, i - 9)
                elif i == 13:
                    emit_update_out(0)

            iters = [(p, b, s) for p in (0, 1) for b in BORDER for s in (0, 1)]
            grp_tiles = [None] * 4
            pend = None
            for i, (p, b, s) in enumerate(iters):
                if pend is None:
                    seps = [[dummy_sep]] * 4
                else:
                    th = pend[1]
                    seps = [th[0:2], th[2:4], th[4:6], th[6:8]]
                pT = emit_scores_exp(p, b, s, seps)
                if pend is not None:
                    emit_azcopy(i - 1, pend[2], grp_tiles)
                if i % 4 == 0 and i > 0:
                    emit_group_norm(i // 4 - 1, grp_tiles)
                AZ, thunks = make_pvz(p, b, s, pT)
                pend = (i, thunks, AZ)
                filler(i)
            for t in pend[1]:
                t()
            emit_azcopy(15, pend[2], grp_tiles)
            emit_group_norm(3, grp_tiles)
            for gi in range(4):
                emit_gate(1, gi)
            emit_update_out(1)

    nc.compile()
    return nc


def _prep_shared(inputs):
    f = np.float32
    c = np.ascontiguousarray
    W_cx, W_ch = np.asarray(inputs["W_cx"], f), np.asarray(inputs["W_ch"], f)
    W_q, W_k, W_v = (np.asarray(inputs[k], f) for k in ("W_q", "W_k", "W_v"))
    W_tok, W_skip = np.asarray(inputs["W_tok"], f), np.asarray(inputs["W_skip"], f)

    blob = np.zeros((128, NWCOL), f)
    blob[0:64, OFF_WIN:OFF_WIN + 128] = np.asarray(inputs["W_in"], f).T
    blob[:, OFF_BIN] = np.asarray(inputs["b_in"], f)
    # conv taps: [c, tap, o]
    blob[:, OFF_CONVH:OFF_CONVH + 1152] = \
        W_ch.transpose(1, 2, 3, 0).reshape(128, 1152)
    blob[:, OFF_CONVX:OFF_CONVX + 1152] = \
        W_cx.transpose(1, 2, 3, 0).reshape(128, 1152)
    # q then k: [c, branch*128]
    blob[:, OFF_QK:OFF_QK + 512] = W_q.transpose(2, 0, 1).reshape(128, 512)
    blob[:, OFF_QK + 512:OFF_QK + 1024] = W_k.transpose(2, 0, 1).reshape(128, 512)
    # wv: [c, src, branch-pair]: xc feeds branches (0,2), hc feeds (1,3)
    blob[:, OFF_WV:OFF_WV + 256] = np.concatenate([W_v[0].T, W_v[2].T], axis=1)
    blob[:, OFF_WV + 256:OFF_WV + 512] = np.concatenate([W_v[1].T, W_v[3].T], axis=1)
    blob[:, OFF_ONES:OFF_ONES + 32] = 1.0
    # tok: [a, gate*4 + branch]
    blob[:, OFF_TOK:OFF_TOK + 2048] = W_tok.transpose(3, 0, 1, 2).reshape(128, 2048)
    # skip: [c, gate*2 + src]
    blob[:, OFF_SKIP:OFF_SKIP + 1024] = \
        W_skip.transpose(3, 0, 1, 2).reshape(128, 1024)
    blob[:, OFF_WOUT:OFF_WOUT + 128] = np.asarray(inputs["W_out"], f).T
    btok = np.asarray(inputs["b_tok"], f).T              # [R, 4]
    sc = np.array([0.5, 0.5, 1.0, 0.5], f)[None, :]
    blob[:, OFF_BTOK:OFF_BTOK + 4] = btok * sc
    blob[:, OFF_BOUT] = np.asarray(inputs["b_out"], f)
    return {"wblob": c(blob)}


def kernel(**inputs):
    from concourse.bass_utils import run_bass_kernel_spmd
    if "nc" not in _CACHE:
        _CACHE["nc"] = _build_program()
    nc = _CACHE["nc"]
    f = np.float32
    x = np.asarray(inputs["x"], f).reshape(N, I, HW)
    hp = np.asarray(inputs["h_prev"], f).reshape(N, R, HW)
    cp = np.asarray(inputs["c_prev"], f).reshape(N, R, HW)
    shared = _prep_shared(inputs)
    in_maps = []
    for ci in range(NCORES):
        sl = slice(S * ci, S * ci + S)
        m = dict(shared)
        m["xin"] = np.ascontiguousarray(x[sl])
        m["hin"] = np.ascontiguousarray(hp[sl])
        m["cin"] = np.ascontiguousarray(cp[sl])
        in_maps.append(m)
    res = run_bass_kernel_spmd(nc, in_maps, core_ids=list(range(NCORES)))
    y = np.concatenate([r["yout"].reshape(S, R, H, W) for r in res.results],
                       axis=0)
    return y.astype(np.float32)
